# revision 61
# baseline (speedup 1.0000x reference)
"""EngramModule kernel for Trainium2 (8 NeuronCores, SPMD data-parallel).

v2 architecture (fp8 DoubleRow 3-term GEMMs + transpose-gather):

Per token t (feature dim H=2048):
  idx[t, h]   = hash of n-gram ending at t (8 heads; computed on host)
  memory[t]   = concat_h tables[h, idx[t, h]]
  key_raw     = memory @ Wk.T ; value_raw = memory @ Wv.T
  gate        = sigmoid(dot(hidden, key_raw)/(sqrt(H)*rms_k) - 4)
  g[t]        = gate * value_raw / rms_v           (value_norm folded in conv w)
  out[t]      = g[t]*(1+w2) + w1*g[t-1] + w0*g[t-2]

Device strategy per core (2048 tokens, 16 tiles of 128):
 - Tables are compacted on host per (core, head) to <=2048 unique rows,
   concatenated into one per-core table, and packed as fp8 e4m3 (hi, lo)
   byte-interleaved 512B rows. dma_gather with transpose=True per
   (token-range, head-group) rectangle delivers memory ALREADY in lhsT
   layout [k-dim on partitions, tokens on free], with (hi, lo) as the two
   bytes of each 16-bit transpose unit -> directly usable as DoubleRow
   operand slots.
 - GEMMs run as 3-term compensated fp8 DoubleRow matmuls (error ~0.1%):
     (Mhi 2-slab)x(Whi 2-slab) + (Mlo)x(Whi) + (Mhi)x(Wlo)
   at 0.25 PE-cycles per output column per 128-contraction.
 - Phase 1 (key): per tile accumulate k in PSUM, ACT-square sumsq + DVE dot
   with bf16 hidden, Newton rsqrt, one ACT Sigmoid -> per-token gate.
 - Phase 2 (value): v in PSUM, sumsq -> rsq_v, g = ACT copy(v * s) bf16;
   g1/g0 cast to fp8; conv = one fp8-DR matmul (shift-pair slots) + one
   fp8-DR boundary matmul per 512-chunk; out = ACT copy(conv psum) + g2.
 - Conv boundary rows for tile 0 are computed EXACTLY on host (2 tokens).
 - Output written bf16, upcast on host.
Only ACT funcs {Square, Sigmoid, Copy} are used -> single act table set,
zero LoadActFuncSet reloads.
"""

import sys

import numpy as np

try:
    import concourse.bass as bass  # noqa: F401
except ImportError:
    sys.path.insert(0, "/opt/trn_rl_repo")

import ml_dtypes

import concourse.bacc as bacc
import concourse.bass as bass
import concourse.tile as tile
from concourse import mybir
from concourse.bass_utils import run_bass_kernel_spmd

E4 = ml_dtypes.float8_e4m3fn
BFNP = ml_dtypes.bfloat16
F32 = mybir.dt.float32
BF16 = mybir.dt.bfloat16
FP8 = mybir.dt.float8e4
I16 = mybir.dt.int16
I32 = mybir.dt.int32
ALU = mybir.AluOpType
AF = mybir.ActivationFunctionType
DR = mybir.MatmulPerfMode.DoubleRow

P = 128
H = 2048
HEADS = 8
HEAD_DIM = 256
VOCAB = 65536
MODULUS = VOCAB - 1
EPS = 1e-6
GATE_BIAS = -4.0
N_CORES = 8
B, S = 4, 4096
TOK = (B * S) // N_CORES        # 2048 tokens per core
NT = TOK // P                   # 16 tiles
CROWS = 2048                    # compact table rows per (core, head)
# gathers batch 2 heads x 256 tokens = 512 indices (safely under the
# 1024-slot SWDGE ring) against the concatenated per-core table [8*2048, 512]
GTOK = 256                      # tokens per gather
NRANGE = TOK // GTOK            # 8 token ranges
NHG = 4                         # head groups of 2
SG = 128.0                      # fp8 scale for g1/g0 (conv operands)
FMAX = 64.0                     # fp8 operand absmax (PSUM partial < ~5.5e4)
NCH = H // 512                  # 4 col chunks of 512


# ---------------------------------------------------------------- host hashing
def _hash_ids_np(ids, mult, off, n):
    """Exact replica of the reference _hash_ids in numpy (wrapping int64)."""
    Bb, Ss = ids.shape
    nh = mult.shape[0]
    ids_u = ids.astype(np.uint64)
    mult_u = mult.astype(np.uint64)
    off_u = off.astype(np.uint64)
    mix = np.zeros((Bb, Ss, nh), dtype=np.uint64)
    for p in range(n):
        shift = n - 1 - p
        tok = np.zeros_like(ids_u)
        if shift > 0:
            tok[:, shift:] = ids_u[:, : Ss - shift]
        else:
            tok = ids_u
        mix ^= tok[:, :, None] * mult_u[None, None, :, p]
    h = (mix + off_u[None, None, :]).view(np.int64)
    hmod = np.remainder(h, MODULUS) + 1
    valid = (np.arange(Ss) >= n - 1)[None, :, None]
    return np.where(valid, hmod, 0)


def _global_indices(input_ids, hm2, ho2, hm3, ho3):
    """[B, S, 8] int32 row indices into the flattened [8*65536, 256] table."""
    h2 = _hash_ids_np(input_ids, hm2, ho2, 2)
    h3 = _hash_ids_np(input_ids, hm3, ho3, 3)
    hid = np.concatenate([h2, h3], axis=-1)          # [B, S, 8]
    gidx = hid + (np.arange(HEADS, dtype=np.int64) * VOCAB)[None, None, :]
    return gidx.astype(np.int32)


def _wrap_idx(inv, n_tok):
    """int16 idx tile [128, n_tok//16]: slot i lives at [i%16, i//16]."""
    t = np.zeros((16, n_tok // 16), np.int16)
    t[np.arange(n_tok) % 16, np.arange(n_tok) // 16] = inv.astype(np.int16)
    return np.ascontiguousarray(np.tile(t, (8, 1)))


# ---------------------------------------------------------------- device program
def build_program():
    nc = bacc.Bacc(None, target_bir_lowering=False)

    t_ctab = nc.dram_tensor("ctab", [HEADS * CROWS, 512], FP8,
                            kind="ExternalInput")
    # idx tiles batched into two tensors: tiny range-0 block loads first so
    # the first gathers start immediately. 64 int16 words per gather.
    IDXW0 = (HEADS // NHG * GTOK) // 16 * NHG
    IDXWR = (HEADS // NHG * GTOK) // 16 * NHG * (NRANGE - 1)
    t_idx0 = nc.dram_tensor("idx0", [P, IDXW0], I16, kind="ExternalInput")
    t_idxr = nc.dram_tensor("idxr", [P, IDXWR], I16, kind="ExternalInput")
    t_wkhi = nc.dram_tensor("wkhi", [P, HEADS, 2, H], FP8, kind="ExternalInput")
    t_wklo = nc.dram_tensor("wklo", [P, HEADS, 2, H], FP8, kind="ExternalInput")
    t_wvhi = nc.dram_tensor("wvhi", [P, HEADS, 2, H], FP8, kind="ExternalInput")
    t_wvlo = nc.dram_tensor("wvlo", [P, HEADS, 2, H], FP8, kind="ExternalInput")
    t_hid = nc.dram_tensor("hid", [TOK, H], BF16, kind="ExternalInput")
    t_w2p1 = nc.dram_tensor("w2p1bc", [P, H], BF16, kind="ExternalInput")
    t_w1s = nc.dram_tensor("w1sbc", [P, H], BF16, kind="ExternalInput")
    t_w0s = nc.dram_tensor("w0sbc", [P, H], BF16, kind="ExternalInput")
    t_s12 = nc.dram_tensor("s12", [P, 2, P], FP8, kind="ExternalInput")
    t_eb = nc.dram_tensor("eb", [P, 2, P], FP8, kind="ExternalInput")
    t_bh = nc.dram_tensor("bhost", [4, H], FP8, kind="ExternalInput")
    t_out = nc.dram_tensor("out", [TOK, H], BF16, kind="ExternalOutput")

    # scalar constants (host-computed, folded scales)
    t_consts = nc.dram_tensor("consts", [P, 4], F32, kind="ExternalInput")
    # consts columns: 0 = eps_k' = (sM*sWk)^2 * H * EPS
    #                 1 = eps_v' = (sM*sWv)^2 * H * EPS
    #                 2 = sqrt(H)
    #                 3 = GATE_BIAS

    with tile.TileContext(nc) as tc:
        with (
            tc.tile_pool(name="pconst", bufs=1) as pc,
            tc.tile_pool(name="pM", bufs=1) as pM,
            tc.tile_pool(name="pWvpre", bufs=1) as pWvpre,
            tc.tile_pool(name="pstat", bufs=4) as pst,
            tc.tile_pool(name="pscr", bufs=2) as pscr,
            tc.tile_pool(name="pbig", bufs=3, space="PSUM") as pbig,
        ):
            # ---- small consts
            idx0_sb = pc.tile([P, IDXW0], I16)
            nc.sync.dma_start(out=idx0_sb[:], in_=t_idx0[:])
            consts = pc.tile([P, 4], F32)
            nc.sync.dma_start(out=consts[:], in_=t_consts[:])
            idxr_sb = pc.tile([P, IDXWR], I16)
            nc.sync.dma_start(out=idxr_sb[:], in_=t_idxr[:])
            s12_sb = pc.tile([P, 2, P], FP8)
            eb_sb = pc.tile([P, 2, P], FP8)
            sg_all = pc.tile([P, NT], F32)      # per-tile gate scalars

            # ---- gathers: (token-range, head-group) rectangles of 1024 idx
            HPG = HEADS // NHG  # heads per gather
            NIG = HPG * GTOK    # idxs per gather
            mseg = [[None] * NHG for _ in range(NRANGE)]

            def emit_gathers(ranges):
                for r in ranges:
                    for hg in range(NHG):
                        g = r * NHG + hg
                        W = NIG // 16
                        if r == 0:
                            iap = idx0_sb[:, hg * W:(hg + 1) * W]
                        else:
                            w0 = (g - NHG) * W
                            iap = idxr_sb[:, w0:w0 + W]
                        m = pM.tile([P, 4 * NIG], FP8, name=f"m{r}_{hg}")
                        nc.gpsimd.dma_gather(
                            out_ap=m[:].rearrange("p (f t) -> p f t", f=4),
                            in_ap=t_ctab[:],
                            idxs_ap=iap,
                            num_idxs=NIG, num_idxs_reg=NIG,
                            elem_size=512, transpose=True)
                        mseg[r][hg] = m

            emit_gathers(range(2))

            def m_slabs(h, i):
                """(lhsT_hi, lhsT_lo) [p, c(2), t(128)] for tile i, head h."""
                m = mseg[i // 2][h // (HEADS // NHG)]
                t0 = (h % (HEADS // NHG)) * GTOK + (i % 2) * P
                ctj = m[:].rearrange("p (c t j) -> p c t j", c=2, t=NIG, j=2)
                return (ctj[:, :, t0:t0 + P, 0], ctj[:, :, t0:t0 + P, 1])

            def gemm_terms(i, whi_sb, wlo_sb, ph, terms, start, stop,
                           chunks=tuple(range(NCH))):
                """Emit a subset of the 3-term fp8 DR GEMM for tile i into
                psum halves ph[0|1]. Terms: 0 = Mhi@Whi, 1 = Mlo@Whi,
                2 = Mhi@Wlo. start/stop apply at the first/last emitted
                matmul of each psum chunk group."""
                for tx, term in enumerate(terms):
                    for h in range(HEADS):
                        hi, lo = m_slabs(h, i)
                        mop = hi if term != 1 else lo
                        wsb = whi_sb if term != 2 else wlo_sb
                        for ch in chunks:
                            pt = ph[ch // 2]
                            cs = slice((ch % 2) * 512, (ch % 2) * 512 + 512)
                            nc.tensor.matmul(
                                pt[:, cs], lhsT=mop,
                                rhs=wsb[:, h, :, ch * 512:(ch + 1) * 512],
                                start=(start and tx == 0 and h == 0),
                                stop=(stop and tx == len(terms) - 1
                                      and h == HEADS - 1),
                                perf_mode=DR)

            def newton_rsqrt(u):
                """In-place u <- 1/sqrt(u) via bit-seed + 2 Newton iters."""
                y = pst.tile([P, 1], F32, tag="ny")
                yi = y[:].bitcast(I32)
                nc.vector.tensor_scalar(out=yi, in0=u[:].bitcast(I32), scalar1=1,
                                        scalar2=None, op0=ALU.logical_shift_right)
                nc.vector.tensor_scalar(out=yi, in0=yi, scalar1=-1,
                                        scalar2=0x5F3759DF, op0=ALU.mult,
                                        op1=ALU.add)
                t2 = pst.tile([P, 1], F32, tag="nt")
                for _ in range(2):
                    nc.vector.tensor_mul(out=t2[:], in0=y[:], in1=y[:])
                    nc.vector.tensor_mul(out=t2[:], in0=t2[:], in1=u[:])
                    nc.vector.tensor_scalar(out=t2[:], in0=t2[:], scalar1=-0.5,
                                            scalar2=1.5, op0=ALU.mult, op1=ALU.add)
                    nc.vector.tensor_mul(out=y[:], in0=y[:], in1=t2[:])
                return y

            # ================= PHASE 1: key =================
            wvlo_sb = pWvpre.tile([P, HEADS, 2, H], FP8)

            def key_half_stats(ph, hid_sb, sq2, dt2, x):
                scr = pscr.tile([P, 1024], F32, tag="scr")
                nc.scalar.activation(out=scr[:], in_=ph[x][:],
                                     func=AF.Square,
                                     accum_out=sq2[:, x:x + 1])
                scr = pscr.tile([P, 1024], F32, tag="scr")
                nc.vector.scalar_tensor_tensor(
                    out=scr[:], in0=ph[x][:], scalar=1.0,
                    in1=hid_sb[:, x * 1024:(x + 1) * 1024],
                    op0=ALU.mult, op1=ALU.mult,
                    accum_out=dt2[:, x:x + 1])

            def key_epi(i, ph, hid_sb, sq2=None, dt2=None):
                if sq2 is None:
                    sq2 = pst.tile([P, 2], F32, tag="sq2")
                    dt2 = pst.tile([P, 2], F32, tag="dt2")
                    for x in range(2):
                        key_half_stats(ph, hid_sb, sq2, dt2, x)
                u = pst.tile([P, 1], F32, tag="u")
                nc.vector.tensor_reduce(out=u[:], in_=sq2[:],
                                        axis=mybir.AxisListType.X, op=ALU.add)
                # u = sumsq + eps_k'  (rsq' = 1/sqrt(u) folds the /sqrt(H))
                nc.vector.tensor_scalar(out=u[:], in0=u[:],
                                        scalar1=consts[:, 0:1], scalar2=None,
                                        op0=ALU.add)
                rsq = newton_rsqrt(u)
                dot = pst.tile([P, 1], F32, tag="dot")
                nc.vector.tensor_reduce(out=dot[:], in_=dt2[:],
                                        axis=mybir.AxisListType.X, op=ALU.add)
                # gate = sigmoid(dot * rsq' - 4)
                nc.scalar.activation(out=sg_all[:, i:i + 1], in_=dot[:],
                                     func=AF.Sigmoid, scale=rsq[:],
                                     bias=consts[:, 3:4])

            with (
                tc.tile_pool(name="pWk", bufs=1) as pWk,
                tc.tile_pool(name="phid", bufs=3) as phid,
                tc.tile_pool(name="pkx", bufs=1, space="PSUM") as pkx,
            ):
                wkhi_sb = pWk.tile([P, HEADS, 2, H], FP8)
                wklo_sb = pWk.tile([P, HEADS, 2, H], FP8)
                for h in range(HEADS):
                    nc.sync.dma_start(out=wkhi_sb[:, h], in_=t_wkhi[:, h])
                # Wk_lo on the Pool queue, sequenced after the range-0/1
                # gathers so the early tiles' gathers win the DMA FIFO
                for h in range(HEADS):
                    nc.gpsimd.dma_start(out=wklo_sb[:, h], in_=t_wklo[:, h])
                emit_gathers(range(2, NRANGE))

                # tiles 0 and 1 are software-pipelined term-wise: their Whi
                # terms run while Wk_lo is still streaming in
                hid01 = []
                ph01 = []
                for i in range(2):
                    hid_sb = phid.tile([P, H], BF16, tag="hid")
                    nc.sync.dma_start(out=hid_sb[:],
                                      in_=t_hid[i * P:(i + 1) * P, :])
                    hid01.append(hid_sb)
                ph01.append([pbig.tile([P, 1024], F32, tag="ps", name="k0_0"),
                             pbig.tile([P, 1024], F32, tag="ps", name="k0_1")])
                ph01.append([pbig.tile([P, 1024], F32, tag="ps", name="k1_0"),
                             pkx.tile([P, 1024], F32, name="k1_1")])
                gemm_terms(0, wkhi_sb, wklo_sb, ph01[0], (0, 1), True, False)
                gemm_terms(1, wkhi_sb, wklo_sb, ph01[1], (0, 1), True, False)
                gemm_terms(0, wkhi_sb, wklo_sb, ph01[0], (2,), False, True)
                key_epi(0, ph01[0], hid01[0])
                gemm_terms(1, wkhi_sb, wklo_sb, ph01[1], (2,), False, True)
                key_epi(1, ph01[1], hid01[1])

                for i in range(2, NT):
                    if i == 8:
                        # mid-key prefetch of Wv_lo (Pool queue; DMA engines
                        # are free of startup traffic by now)
                        for h in range(HEADS):
                            nc.gpsimd.dma_start(out=wvlo_sb[:, h],
                                                in_=t_wvlo[:, h])
                    hid_sb = phid.tile([P, H], BF16, tag="hid")
                    nc.sync.dma_start(out=hid_sb[:],
                                      in_=t_hid[i * P:(i + 1) * P, :])
                    ph = [pbig.tile([P, 1024], F32, tag="ps", name=f"k{i}_{x}")
                          for x in range(2)]
                    sq2 = pst.tile([P, 2], F32, tag="sq2")
                    dt2 = pst.tile([P, 2], F32, tag="dt2")
                    # half-major: half 0's stats overlap half 1's GEMM
                    for x in range(2):
                        gemm_terms(i, wkhi_sb, wklo_sb, ph, (0, 1, 2),
                                   True, True, chunks=(2 * x, 2 * x + 1))
                        key_half_stats(ph, hid_sb, sq2, dt2, x)
                    key_epi(i, ph, hid_sb, sq2, dt2)

            # ================= PHASE 2: value =================
            with (
                tc.tile_pool(name="pWvhi", bufs=1) as pWvhi,
                tc.tile_pool(name="pg", bufs=2) as pg,
                tc.tile_pool(name="pout", bufs=2) as pout,
                tc.tile_pool(name="pb", bufs=1) as pb,
                tc.tile_pool(name="pconv", bufs=2, space="PSUM") as pconv,
            ):
                # Wv_hi streamed on SP first; the small constant tiles after
                # it so they don't steal DMA-engine slots from the stream
                wvhi_sb = pWvhi.tile([P, HEADS, 2, H], FP8)
                for h in range(HEADS):
                    nc.sync.dma_start(out=wvhi_sb[:, h], in_=t_wvhi[:, h])
                w2p1_sb = pWvhi.tile([P, H], BF16)
                w1s_sb = pWvhi.tile([P, H], BF16)
                w0s_sb = pWvhi.tile([P, H], BF16)
                nc.sync.dma_start(out=w2p1_sb[:], in_=t_w2p1[:])
                nc.sync.dma_start(out=w1s_sb[:], in_=t_w1s[:])
                nc.sync.dma_start(out=w0s_sb[:], in_=t_w0s[:])
                nc.sync.dma_start(out=s12_sb[:], in_=t_s12[:])
                nc.sync.dma_start(out=eb_sb[:], in_=t_eb[:])

                # boundary ping-pong tiles (fixed, fully memset once so reads
                # of untouched rows are well-defined); tile 0's rows from host
                b_tiles = [pb.tile([P, 2, H], FP8, name=f"b{x}")
                           for x in range(2)]
                nc.vector.memset(b_tiles[0][:], 0.0)
                nc.vector.memset(b_tiles[1][:], 0.0)
                nc.sync.dma_start(out=b_tiles[0][0:4, 0, :], in_=t_bh[:])

                for i in range(NT):
                    b_prev = b_tiles[i % 2]
                    ph = [pbig.tile([P, 1024], F32, tag="ps", name=f"v{i}_{x}")
                          for x in range(2)]
                    sq2 = pst.tile([P, 2], F32, tag="sq2")
                    # half-major: half 0's sumsq overlaps half 1's GEMM
                    for x in range(2):
                        gemm_terms(i, wvhi_sb, wvlo_sb, ph, (2, 0, 1),
                                   True, True, chunks=(2 * x, 2 * x + 1))
                        scr = pscr.tile([P, 1024], F32, tag="scr")
                        nc.scalar.activation(out=scr[:], in_=ph[x][:],
                                             func=AF.Square,
                                             accum_out=sq2[:, x:x + 1])
                    u = pst.tile([P, 1], F32, tag="u")
                    nc.vector.tensor_reduce(out=u[:], in_=sq2[:],
                                            axis=mybir.AxisListType.X,
                                            op=ALU.add)
                    nc.vector.tensor_scalar(out=u[:], in0=u[:],
                                            scalar1=consts[:, 1:2], scalar2=None,
                                            op0=ALU.add)
                    rsq = newton_rsqrt(u)
                    # s_final = rsq * sqrt(H) * gate
                    sfin = pst.tile([P, 1], F32, tag="sfin")
                    nc.vector.scalar_tensor_tensor(
                        out=sfin[:], in0=rsq[:], scalar=consts[:, 2:3],
                        in1=sg_all[:, i:i + 1], op0=ALU.mult, op1=ALU.mult)
                    # per-half epilogue pipeline: g -> g2/g10 -> conv -> copy
                    # -> add -> out DMA, so half 0 drains while half 1 is
                    # still in its GEMM/stat stages (shortens the end tail)
                    g = pg.tile([P, H], BF16, tag="g")
                    g2 = pg.tile([P, H], BF16, tag="g2")
                    g10 = pg.tile([P, 2, H], FP8, tag="g10")
                    out_sb = pout.tile([P, H], BF16, tag="out")
                    for x in range(2):
                        hs = slice(x * 1024, (x + 1) * 1024)
                        nc.scalar.activation(out=g[:, hs], in_=ph[x][:],
                                             func=AF.Copy, scale=sfin[:])
                        nc.vector.tensor_mul(out=g10[:, 0, hs], in0=g[:, hs],
                                             in1=w1s_sb[:, hs])
                        nc.vector.tensor_mul(out=g10[:, 1, hs], in0=g[:, hs],
                                             in1=w0s_sb[:, hs])
                        nc.vector.tensor_mul(out=g2[:, hs], in0=g[:, hs],
                                             in1=w2p1_sb[:, hs])
                        for ch in (2 * x, 2 * x + 1):
                            cs = slice(ch * 512, (ch + 1) * 512)
                            pcv = pconv.tile([P, 512], F32, tag="pcv")
                            nc.tensor.matmul(pcv[:], lhsT=s12_sb[:],
                                             rhs=g10[:, :, cs],
                                             start=True, stop=False,
                                             perf_mode=DR)
                            nc.tensor.matmul(pcv[:], lhsT=eb_sb[:],
                                             rhs=b_prev[:, :, cs],
                                             start=False, stop=True,
                                             perf_mode=DR)
                            nc.scalar.activation(out=out_sb[:, cs], in_=pcv[:],
                                                 func=AF.Copy)
                        nc.vector.tensor_add(out=out_sb[:, hs],
                                             in0=out_sb[:, hs],
                                             in1=g2[:, hs])
                        nc.sync.dma_start(out=t_out[i * P:(i + 1) * P, hs],
                                          in_=out_sb[:, hs])

                    # boundary rows for next tile from g10 tails
                    if i < NT - 1:
                        b_next = b_tiles[(i + 1) % 2]
                        nc.sync.dma_start(out=b_next[0:1, 0, :],
                                          in_=g10[127:128, 0, :])
                        nc.sync.dma_start(out=b_next[1:3, 0, :],
                                          in_=g10[126:128, 1, :])

    nc.compile()
    return nc


# ---------------------------------------------------------------- host wrapper
_PROGRAM = None


def _get_program():
    global _PROGRAM
    if _PROGRAM is None:
        _PROGRAM = build_program()
    return _PROGRAM


def kernel(hidden_states, input_ids, tables, Wk, Wv, key_norm_w, value_norm_w,
           conv_w, hm2, ho2, hm3, ho3):
    hidden_states = np.asarray(hidden_states, dtype=np.float32)
    input_ids = np.asarray(input_ids, dtype=np.int64)
    tables = np.asarray(tables, dtype=np.float32)
    Wk = np.asarray(Wk, dtype=np.float32)
    Wv = np.asarray(Wv, dtype=np.float32)
    key_norm_w = np.asarray(key_norm_w, dtype=np.float32)
    value_norm_w = np.asarray(value_norm_w, dtype=np.float32)
    conv_w = np.asarray(conv_w, dtype=np.float32)

    gidx = _global_indices(input_ids, np.asarray(hm2), np.asarray(ho2),
                           np.asarray(hm3), np.asarray(ho3))   # [B,S,8] i32
    hid_local = (gidx % VOCAB).reshape(B * S, HEADS)           # per-head rows

    # fold key_norm into hidden; fp8 scales
    hid_flat = (hidden_states.reshape(B * S, H) * key_norm_w[None, :])
    hid_bf = hid_flat.astype(BFNP)

    sM = FMAX / max(np.abs(tables).max(), 1e-30)
    sWk = FMAX / max(np.abs(Wk).max(), 1e-30)
    sWv = FMAX / max(np.abs(Wv).max(), 1e-30)

    def split_fp8(x, s):
        hi = (x * s).astype(E4)
        lo = ((x * s) - hi.astype(np.float32)).astype(E4)
        return hi, lo

    def w_layout(W, s):
        # [p, head, c, n] with W.T[k, n] = W[n, k]; k = h*256 + c*128 + p
        hi, lo = split_fp8(np.ascontiguousarray(W.T), s)   # [k, n]
        def lay(a):
            return np.ascontiguousarray(
                a.reshape(HEADS, 2, P, H).transpose(2, 0, 1, 3))
        return lay(hi), lay(lo)

    wkhi, wklo = w_layout(Wk, sWk)
    wvhi, wvlo = w_layout(Wv, sWv)

    # conv weight foldings (value_norm + fp8 g-scale)
    w0 = conv_w[:, 0] * value_norm_w
    w1 = conv_w[:, 1] * value_norm_w
    w2p1 = (1.0 + conv_w[:, 2]) * value_norm_w
    w2p1bc = np.ascontiguousarray(np.broadcast_to(w2p1, (P, H))).astype(BFNP)
    w1sbc = np.ascontiguousarray(np.broadcast_to(w1 * SG, (P, H))).astype(BFNP)
    w0sbc = np.ascontiguousarray(np.broadcast_to(w0 * SG, (P, H))).astype(BFNP)

    # shift-pair (s1, s2) and boundary lhsT matrices, scaled by 1/SG
    inv = np.float32(1.0 / SG)
    s12 = np.zeros((P, 2, P), E4)
    s12[:, 0, :] = (np.eye(P, k=1, dtype=np.float32) * inv).astype(E4)
    s12[:, 1, :] = (np.eye(P, k=2, dtype=np.float32) * inv).astype(E4)
    eb = np.zeros((P, 2, P), E4)
    # b_pad rows: 0 -> out0 (g1[127]), 1 -> out0 (g0[126]), 2 -> out1 (g0[127])
    eb[0, 0, 0] = E4(inv)
    eb[1, 0, 0] = E4(inv)
    eb[2, 0, 1] = E4(inv)

    consts = np.zeros((P, 4), np.float32)
    consts[:, 0] = (sM * sWk) ** 2 * H * EPS
    consts[:, 1] = (sM * sWv) ** 2 * H * EPS
    consts[:, 2] = np.sqrt(np.float32(H))
    consts[:, 3] = GATE_BIAS

    # exact host reference for the 2 boundary tokens of each core
    def host_gated(trange):
        """gated[t] rows (f64->f32) for global token indices trange."""
        out = np.zeros((len(trange), H), np.float32)
        tabs = tables.astype(np.float64)
        for j, t in enumerate(trange):
            rows = [tabs[h, hid_local[t, h]] for h in range(HEADS)]
            mem = np.concatenate(rows)                     # [2048]
            kr = mem @ Wk.T.astype(np.float64)
            vr = mem @ Wv.T.astype(np.float64)
            rk = 1.0 / np.sqrt(np.mean(kr ** 2) + EPS)
            rv = 1.0 / np.sqrt(np.mean(vr ** 2) + EPS)
            z = float(hid_flat[t].astype(np.float64) @ (kr * rk)) / np.sqrt(H) \
                + GATE_BIAS
            gate = 1.0 / (1.0 + np.exp(-z))
            out[j] = (gate * (vr * rv) * value_norm_w).astype(np.float32)
        return out

    in_maps = []
    for r in range(N_CORES):
        t0 = r * TOK
        idx_core = hid_local[t0:t0 + TOK]                  # [2048, 8]
        in_map = {}
        packed = np.zeros((HEADS * CROWS, 512), np.uint8)
        gidx16 = np.empty((TOK, HEADS), np.int16)          # h*2048 + inv
        for h in range(HEADS):
            uniq, invmap = np.unique(idx_core[:, h], return_inverse=True)
            rows = tables[h, uniq]                         # [n_u, 256]
            hi, lo = split_fp8(rows, sM)
            blk = packed[h * CROWS:h * CROWS + len(uniq)]
            blk[:, 0::2] = hi.view(np.uint8)
            blk[:, 1::2] = lo.view(np.uint8)
            gidx16[:, h] = (h * CROWS + invmap).astype(np.int16)
        in_map["ctab"] = packed.view(E4)
        # gather idx rectangles: (token-range r, head-group hg): the 1024
        # idxs are head-major (4 heads x 256 tokens) to match m_slabs
        idx_cols = []
        for r_ in range(NRANGE):
            for hg in range(NHG):
                hpg = HEADS // NHG
                rect = gidx16[r_ * GTOK:(r_ + 1) * GTOK,
                              hg * hpg:(hg + 1) * hpg]
                seq = np.ascontiguousarray(rect.T).reshape(-1)   # head-major
                idx_cols.append(_wrap_idx(seq, hpg * GTOK))
        in_map["idx0"] = np.ascontiguousarray(
            np.concatenate(idx_cols[:NHG], axis=1))
        in_map["idxr"] = np.ascontiguousarray(
            np.concatenate(idx_cols[NHG:], axis=1))

        # boundary rows for tile 0
        bh = np.zeros((4, H), np.float32)
        if t0 % S != 0:
            gtwo = host_gated([t0 - 1, t0 - 2])            # [2, H]
            bh[0] = gtwo[0] * w1 * SG                      # g1[t0-1]
            bh[1] = gtwo[1] * w0 * SG                      # g0[t0-2]
            bh[2] = gtwo[0] * w0 * SG                      # g0[t0-1]
        in_map["bhost"] = bh.astype(E4)

        in_map.update({
            "wkhi": wkhi, "wklo": wklo, "wvhi": wvhi, "wvlo": wvlo,
            "hid": np.ascontiguousarray(hid_bf[t0:t0 + TOK]),
            "w2p1bc": w2p1bc, "w1sbc": w1sbc, "w0sbc": w0sbc,
            "s12": s12, "eb": eb, "consts": consts,
        })
        in_maps.append(in_map)

    nc = _get_program()
    res = run_bass_kernel_spmd(nc, in_maps, list(range(N_CORES)))
    out = np.empty((B * S, H), np.float32)
    for r in range(N_CORES):
        out[r * TOK:(r + 1) * TOK] = res.results[r]["out"].astype(np.float32)
    return out.reshape(B, S, H)
